# revision 1
# baseline (speedup 1.0000x reference)
import sys
sys.path.insert(0, '/opt/trn_rl_repo')
import numpy as np
import concourse.bass as bass
import concourse.bacc as bacc
import concourse.mybir as mybir
from concourse import bass_utils
from concourse.tile import TileContext

S = 2048          # states
T = 4096          # sequence length
SC = 16           # state chunks of 128
DT = mybir.dt.float16
NPDT = mybir.dt.np(DT)
NCHUNK = 8        # B streamed in NCHUNK pieces
STEPS_PER_CHUNK = T // NCHUNK

_cache = {}


def _build(total_steps=T - 1):
    key = ('nc', total_steps)
    if key in _cache:
        return _cache[key]
    nc = bacc.Bacc(None)
    A_d = nc.dram_tensor("Ablk", [128, SC * SC * 128], DT, kind="ExternalInput")
    B_d = nc.dram_tensor("Bcols", [128, T * SC], DT, kind="ExternalInput")
    a0_d = nc.dram_tensor("alpha0", [128, SC], DT, kind="ExternalInput")
    out_d = nc.dram_tensor("out", [1, 1], mybir.dt.float32, kind="ExternalOutput")

    with TileContext(nc) as tc:
        with (
            tc.tile_pool(name="main", bufs=1) as pool,
            tc.tile_pool(name="bstream", bufs=2) as bpool,
            tc.tile_pool(name="ps", bufs=1, space="PSUM") as pspool,
        ):
            A_sb = pool.tile([128, SC * SC * 128], DT, tag="A")
            alpha = pool.tile([128, SC], DT, tag="alpha")
            ps = pspool.tile([128, SC], mybir.dt.float32, tag="ps")
            nc.sync.dma_start(A_sb[:], A_d[:])
            nc.sync.dma_start(alpha[:], a0_d[:])
            w = STEPS_PER_CHUNK * SC
            done = 0
            for q in range(NCHUNK):
                if done >= total_steps:
                    break
                Bbuf = bpool.tile([128, w], DT, tag="B")
                nc.sync.dma_start(Bbuf[:], B_d[:, q * w:(q + 1) * w])
                nsteps = min(STEPS_PER_CHUNK, total_steps - done)
                done += nsteps
                with tc.For_i(0, nsteps, 1) as i:
                    for d in range(SC):
                        for c in range(SC):
                            nc.tensor.matmul(
                                ps[:, d:d + 1],
                                A_sb[:, (c * SC + d) * 128:(c * SC + d + 1) * 128],
                                alpha[:, c:c + 1],
                                start=(c == 0),
                                stop=(c == SC - 1),
                            )
                    nc.vector.tensor_mul(alpha[:, :], ps[:, :], Bbuf[:, bass.ts(i, SC)])
            ones = pool.tile([128, 1], DT, tag="ones")
            nc.vector.memset(ones[:], 1.0)
            ps2 = pspool.tile([1, SC], mybir.dt.float32, tag="ps2")
            nc.tensor.matmul(ps2[:], ones[:], alpha[:], start=True, stop=True)
            red = pool.tile([1, 1], mybir.dt.float32, tag="red")
            nc.vector.reduce_sum(red[:], ps2[:], axis=mybir.AxisListType.X)
            nc.sync.dma_start(out_d[:], red[:])
    nc.finalize()
    _cache[key] = nc
    return nc


def _prep_inputs(observations, A, B, pi):
    obs = np.asarray(observations).astype(np.int64)
    A = np.asarray(A, dtype=np.float32)
    B = np.asarray(B, dtype=np.float32)
    pi = np.asarray(pi, dtype=np.float32)
    B_obs = B[:, obs].T            # [T, S]
    alpha0 = pi * B_obs[0]         # [S]
    # A block layout: Ablk[p, ((c*SC+d)*128)+m] = A[128c+p, 128d+m]
    Ablk = np.ascontiguousarray(
        A.reshape(SC, 128, SC, 128).transpose(1, 0, 2, 3).reshape(128, -1)
    ).astype(NPDT)
    # b columns for steps 1..T-1 (+1 zero pad), layout [p, t*SC + c] = B_obs[t][128c+p]
    Bsteps = np.zeros((T, S), np.float32)
    Bsteps[:T - 1] = B_obs[1:]
    Bcols = np.ascontiguousarray(
        Bsteps.reshape(T, SC, 128).transpose(2, 0, 1).reshape(128, T * SC)
    ).astype(NPDT)
    a0col = np.ascontiguousarray(alpha0.reshape(SC, 128).T).astype(NPDT)
    return {"Ablk": Ablk, "Bcols": Bcols, "alpha0": a0col}


def kernel(observations, A, B, pi, _want_results=False):
    nc = _build()
    in_map = _prep_inputs(observations, A, B, pi)
    in_maps = [dict(in_map) for _ in range(8)]
    res = bass_utils.run_bass_kernel_spmd(nc, in_maps, core_ids=list(range(8)))
    out = np.float32(res.results[0]["out"][0, 0])
    if _want_results:
        return out, res
    return np.asarray(out, dtype=np.float32)

